# revision 1
# baseline (speedup 1.0000x reference)
"""EGNN (gnn_message_passing) Trainium2 Bass kernel, v2.

Restructured from the v1 feature-major pipeline around three cost-model
facts: fp32 matmuls run 4 cycles/row vs bf16's 1; Silu and Sigmoid live
in different ACT table sets (1.3us load per switch) while Tanh shares
Silu's set; and ACT/DVE cost scales with free size only, so work should
sit on the partition axis whenever possible.

Layout: chunks are j-PAIRS (2 source nodes x all 256 i), with the two
parities in SBUF partition quadrants 0:50 and 64:114 (engine access
patterns must start at 32-aligned partitions). Per chunk:
- stage1 h_pre [128,256]: per-i term re-added via a padded-identity
  matmul against the per-layer fiW, dist term as a K=2 matmul against
  host-precomputed rel_dist rows; the per-j term W1fj.f_j+b1 is a
  per-partition ACT bias, so silu50 is ONE [128,256] op per 512 edges.
- stage2 is transposed: lhsT=h quadrant slices, rhs=W2 (replicated in
  both quadrants) -> m_pre edge-major [128i, 64f] blocks packed
  8-per-PSUM-bank; b2 enters via K=1 ones matmuls; silu64 is ONE
  [128,512] op per 2 chunks.
- gate: sigmoid(z) = 0.5*tanh(z/2)+0.5 (tanh shares Silu's ACT table;
  the 0.5 folds into node_w1). z = per-block reduce of m*w_g; gating
  and j-aggregation are per-partition-scalar multiply-adds on DVE/Pool.
- aggregates [128i,64f] transpose back to [64,256] via PE at layer end.
All matmul operands are bf16 (PSUM accumulation stays fp32).
"""

import numpy as np
import ml_dtypes

import concourse.bass as bass
import concourse.bacc as bacc
import concourse.mybir as mybir
from concourse.tile import TileContext
from concourse.bass_utils import run_bass_kernel_spmd

F32 = mybir.dt.float32
F32R = mybir.dt.float32r
BF16 = mybir.dt.bfloat16
AF = mybir.ActivationFunctionType
ALU = mybir.AluOpType
X = mybir.AxisListType.X

LIP = 0.909
NCORES = 8
BM = 2            # molecules per core
N = 256           # nodes per molecule
L = 2             # layers
D = 12            # feature dim
M = 64            # message dim
EH = 50           # edge hidden
Q = 64            # partition quadrant stride for the j-odd half
NCHUNK = 128      # j-pair chunks per molecule-layer
NMEB = 10         # m_em sbuf ring depth

WBF_SPEC = [
    ("w1fi", D, L * EH), ("w1fjE", D, L * 128), ("w1fjO", D, L * 128),
    ("b1pad", 1, L * 128), ("w1d4q", 4, L * 128), ("w2q", 128, L * M),
    ("b2row", 1, L * M), ("wgrep", 128, L * 512), ("ipad", EH, 128),
    ("i128b", 128, 128), ("c12", D, 1),
]
WF32_SPEC = [
    ("gbh", 128, L), ("lng", D, L), ("lnb", D, L), ("nw1a", D, L * 24),
    ("nw1b", M, L * 24), ("nb1", 24, L), ("nw2", 24, L * D),
    ("nb2", D, L), ("mw1", D, M), ("mb1", M, 1), ("mw2", M, 2),
    ("mb2", 2, 1), ("i128f", 128, 128),
]


def _offsets(spec):
    out, off = {}, 0
    for nm, p, w in spec:
        out[nm] = (p, w, off)
        off += w
    return out, off


WBF_OFF, WBF_X = _offsets(WBF_SPEC)
WF32_OFF, WF32_X = _offsets(WF32_SPEC)


def build_nc():
    nc = bacc.Bacc("TRN2", target_bir_lowering=False, debug=False)

    feats0 = nc.dram_tensor("feats0", [BM, D, N], F32, kind="ExternalInput")
    rdin = nc.dram_tensor("rdin", [BM, 4, 128, N], BF16, kind="ExternalInput")
    mask12 = nc.dram_tensor("mask12", [BM, D, N], F32, kind="ExternalInput")
    wbf = nc.dram_tensor("wbf", [128, WBF_X], BF16, kind="ExternalInput")
    wf32 = nc.dram_tensor("wf32", [128, WF32_X], F32, kind="ExternalInput")
    out = nc.dram_tensor("out", [BM, N, 2, 6], F32, kind="ExternalOutput")

    with TileContext(nc) as tc:
        with (
            tc.tile_pool(name="singles", bufs=1) as S,
            tc.tile_pool(name="mol", bufs=3) as MP,
            tc.tile_pool(name="rdp", bufs=1) as RDP,
            tc.tile_pool(name="lay", bufs=3) as LP,
            tc.tile_pool(name="ph", bufs=3, space="PSUM") as PH,
            tc.tile_pool(name="pm", bufs=2, space="PSUM") as PM,
            tc.tile_pool(name="psml", bufs=2, space="PSUM") as PS,
            tc.tile_pool(name="pg", bufs=1, space="PSUM") as PG,
        ):
            def ld(dram, p, f, nm, dt=BF16):
                t = S.tile([p, f], dt, tag=nm, name=nm)
                nc.sync.dma_start(out=t, in_=dram[:, :])
                return t

            wbf_s = S.tile([128, WBF_X], BF16, tag="wbf", name="wbf")
            nc.sync.dma_start(out=wbf_s, in_=wbf[:, :])
            wf32_s = S.tile([128, WF32_X], F32, tag="wf32", name="wf32")
            nc.sync.dma_start(out=wf32_s, in_=wf32[:, :])

            def bsl(nm):
                p, w, off = WBF_OFF[nm]
                return wbf_s[0:p, off:off + w]

            def fsl(nm):
                p, w, off = WF32_OFF[nm]
                return wf32_s[0:p, off:off + w]

            w1fi_s = bsl("w1fi")
            w1fjE_s = bsl("w1fjE")
            w1fjO_s = bsl("w1fjO")
            b1pad_s = bsl("b1pad")
            w1d4q_s = bsl("w1d4q")
            w2q_s = bsl("w2q")
            b2row_s = bsl("b2row")
            wgrep_s = bsl("wgrep")
            ipad_s = bsl("ipad")
            i128b_s = bsl("i128b")
            c12_s = bsl("c12")
            gbh_s = fsl("gbh")
            lng_s = fsl("lng")
            lnb_s = fsl("lnb")
            nw1a_s = fsl("nw1a")
            nw1b_s = fsl("nw1b")
            nb1_s = fsl("nb1")
            nw2_s = fsl("nw2")
            nb2_s = fsl("nb2")
            mw1_s = fsl("mw1")
            mb1_s = fsl("mb1")
            mw2_s = fsl("mw2")
            mb2_s = fsl("mb2")
            i128f_s = fsl("i128f")

            onesr = S.tile([1, 128], BF16, tag="onesr")
            nc.vector.memset(onesr, 1.0)
            eps = S.tile([1, 1], F32, tag="eps")
            nc.vector.memset(eps, 1e-5)

            h_bufs = [S.tile([128, 512], BF16, tag=f"h{k}", name=f"h{k}")
                      for k in range(3)]
            me_bufs = [S.tile([128, 512], BF16, tag=f"me{k}", name=f"me{k}")
                       for k in range(NMEB)]
            tmp_bufs = [S.tile([128, 512], BF16, tag=f"tmp{k}",
                               name=f"tmp{k}") for k in range(3)]
            zb_bufs = [S.tile([128, 64], F32, tag=f"zb{k}", name=f"zb{k}")
                       for k in range(2)]
            th_bufs = [S.tile([128, 64], BF16, tag=f"th{k}", name=f"th{k}")
                       for k in range(2)]
            tp_bufs = [S.tile([128, 64], BF16, tag=f"tp{k}", name=f"tp{k}")
                       for k in range(2)]
            opad = S.tile([2, N, 6], F32, tag="opad")
            nc.vector.memset(opad, 0.0)

            gcount = 0   # global 2-chunk group counter
            scount = 0   # global super-group (8 chunks) counter

            for mol in range(BM):
                feats = MP.tile([D, N], F32, tag="feats")
                nc.sync.dma_start(out=feats, in_=feats0[mol])
                rd2 = RDP.tile([4, 128, N], BF16, tag="rd2")
                nc.sync.dma_start(out=rd2, in_=rdin[mol])
                msk = MP.tile([D, N], F32, tag="msk")
                nc.sync.dma_start(out=msk, in_=mask12[mol])

                for lay in range(L):
                    fb = LP.tile([D, N], BF16, tag="fb")
                    nc.vector.tensor_copy(out=fb, in_=feats)
                    # ---- per-layer fiW and quadrant-stacked fjwb ----
                    ps_fi = PS.tile([EH, N], F32, tag="pa")
                    nc.tensor.matmul(
                        ps_fi, lhsT=w1fi_s[:, lay * EH:(lay + 1) * EH],
                        rhs=fb, start=True, stop=True)
                    fiW = LP.tile([EH, N], BF16, tag="fiW")
                    nc.vector.tensor_copy(out=fiW, in_=ps_fi)

                    ps_fj = PS.tile([128, 128], F32, tag="pa")
                    fe = fb.rearrange("p (c two) -> p two c", two=2)
                    nc.tensor.matmul(
                        ps_fj, lhsT=w1fjE_s[:, lay * 128:(lay + 1) * 128],
                        rhs=fe[:, 0, :], start=True, stop=False)
                    nc.tensor.matmul(
                        ps_fj, lhsT=w1fjO_s[:, lay * 128:(lay + 1) * 128],
                        rhs=fe[:, 1, :], start=False, stop=False)
                    nc.tensor.matmul(
                        ps_fj, lhsT=b1pad_s[:, lay * 128:(lay + 1) * 128],
                        rhs=onesr, start=False, stop=True)
                    fjwb = LP.tile([128, 128], F32, tag="fjwb")
                    nc.vector.tensor_copy(out=fjwb, in_=ps_fj)
                    ps_ft = PS.tile([128, 128], F32, tag="pa")
                    nc.tensor.transpose(ps_ft, fjwb, i128f_s)
                    fjT = LP.tile([128, 128], BF16, tag="fjT")
                    nc.vector.tensor_copy(out=fjT, in_=ps_ft)

                    w1d4q_l = w1d4q_s[:, lay * 128:(lay + 1) * 128]
                    w2q_l = w2q_s[:, lay * M:(lay + 1) * M]
                    b2_l = b2row_s[:, lay * M:(lay + 1) * M]
                    wg_l = wgrep_s[:, lay * 512:(lay + 1) * 512]
                    ps_mg = PG.tile([M, N], F32, tag="magg")

                    for cc in range(NCHUNK // 2):
                        c = 2 * cc
                        # ---- stage 1: h_pre [128, 512] = 2 chunks ----
                        ph = PH.tile([128, 512], F32, tag="hpre")
                        for half in range(2):
                            sl = slice(half * N, (half + 1) * N)
                            nc.tensor.matmul(ph[:, sl], lhsT=ipad_s, rhs=fiW,
                                             start=True, stop=False)
                            nc.tensor.matmul(
                                ph[:, sl], lhsT=w1d4q_l,
                                rhs=rd2[:, c + half, :],
                                start=False, stop=False)
                            icol = i128b_s[:, 2 * cc + half:
                                           2 * cc + half + 1]
                            ind_ap = bass.AP(
                                tensor=icol.tensor, offset=icol.offset,
                                ap=[list(icol.ap[0]), [0, N]])
                            nc.tensor.matmul(ph[:, sl], lhsT=fjT,
                                             rhs=ind_ap,
                                             start=False, stop=True)
                        h = h_bufs[cc % 3]
                        nc.scalar.activation(h, ph, AF.Silu)

                        # ---- stage 2 (transposed): m_pre [128, 64] x8 ----
                        pm = PM.tile([128, 512], F32, tag="mpre")
                        for half in range(2):
                            for jj in range(2):   # i-parity quadrant
                                for ib in range(2):   # j half-block
                                    b = half * 4 + jj * 2 + ib
                                    po = pm[:, b * M:(b + 1) * M]
                                    hs = h[jj * Q:jj * Q + EH,
                                           half * N + ib * 128:
                                           half * N + (ib + 1) * 128]
                                    w2s = w2q_l[jj * Q:jj * Q + EH, :]
                                    nc.tensor.matmul(po, lhsT=hs, rhs=w2s,
                                                     start=True, stop=False)
                                    nc.tensor.matmul(po, lhsT=onesr,
                                                     rhs=b2_l,
                                                     start=False, stop=True)

                        g = gcount
                        me = me_bufs[g % NMEB]
                        nc.scalar.activation(me, pm, AF.Silu)
                        tmp = tmp_bufs[g % 3]
                        nc.vector.tensor_mul(tmp, me, wg_l)
                        zb = zb_bufs[(g // 8) % 2]
                        nc.vector.tensor_reduce(
                            out=zb[:, (g % 8) * 8:(g % 8) * 8 + 8],
                            in_=tmp.rearrange("p (b f) -> p b f", b=8),
                            op=ALU.add, axis=X)
                        gcount += 1

                        if cc == 8:
                            # ---- LayerNorm of feats (feeds node MLP later) ----
                            # var = E[(x-mu)^2]; mu/rstd broadcast via f32r matmuls
                            # (exact fp32 in sim, 1 cycle/row) so low-variance nodes
                            # stay accurate and var is nonnegative.
                            ps_mu = PS.tile([1, N], F32, tag="pa")
                            nc.tensor.matmul(ps_mu, lhsT=c12_s, rhs=fb,
                                             start=True, stop=True)
                            stat = LP.tile([1, 2 * N], BF16, tag="stat")
                            nc.vector.tensor_copy(out=stat[:, 0:N], in_=ps_mu)
                            ps_bm = PS.tile([D, N], F32, tag="pa")
                            nc.tensor.matmul(ps_bm, lhsT=onesr[:, 0:D],
                                             rhs=stat[:, 0:N], start=True, stop=True)
                            ctr = LP.tile([D, N], BF16, tag="ctr")
                            nc.vector.tensor_sub(ctr, fb, ps_bm)
                            sqc = LP.tile([D, N], BF16, tag="sqc")
                            nc.vector.tensor_mul(sqc, ctr, ctr)
                            ps_v = PS.tile([1, N], F32, tag="pa")
                            nc.tensor.matmul(ps_v, lhsT=c12_s, rhs=sqc,
                                             start=True, stop=True)
                            sd = LP.tile([1, N], F32, tag="sd")
                            nc.scalar.activation(sd, ps_v, AF.Sqrt, bias=eps[:, 0:1])
                            with nc.allow_low_precision(reason="bf16 rstd"):
                                nc.vector.reciprocal(stat[:, N:2 * N], sd)
                            ps_br = PS.tile([D, N], F32, tag="pa")
                            nc.tensor.matmul(ps_br, lhsT=onesr[:, 0:D],
                                             rhs=stat[:, N:2 * N], start=True,
                                             stop=True)
                            nrm = LP.tile([D, N], BF16, tag="nrm")
                            nc.vector.tensor_mul(nrm, ctr, ps_br)
                            normed = LP.tile([D, N], F32, tag="normed")
                            nc.vector.tensor_scalar(
                                out=normed, in0=nrm,
                                scalar1=lng_s[:, lay:lay + 1],
                                scalar2=lnb_s[:, lay:lay + 1],
                                op0=ALU.mult, op1=ALU.add)


                        if cc % 8 == 7:
                            s = scount
                            zb = zb_bufs[s % 2]
                            th = th_bufs[s % 2]
                            nc.scalar.activation(
                                th, zb, AF.Tanh,
                                bias=gbh_s[:, lay:lay + 1], scale=0.5)
                            tp = tp_bufs[s % 2]
                            nc.vector.tensor_scalar_add(tp, th, 1.0)
                            # aggregate the 8 groups of this super-group:
                            # magg[:, i] accumulates me_blk.T @ tp1_col
                            g0 = gcount - 8
                            for q in range(8):
                                me = me_bufs[(g0 + q) % NMEB]
                                for bb in range(8):
                                    jb = bb % 2
                                    ip = (bb // 2) % 2
                                    par = bb // 4
                                    i = 4 * ((g0 + q) % 64) + 2 * par + ip
                                    col = q * 8 + bb
                                    nc.tensor.matmul(
                                        ps_mg[:, i:i + 1],
                                        lhsT=me[:, bb * M:(bb + 1) * M],
                                        rhs=tp[:, col:col + 1],
                                        start=(jb == 0), stop=(jb == 1))
                            scount += 1

                    magg = LP.tile([M, N], F32, tag="magg")
                    nc.vector.tensor_copy(out=magg, in_=ps_mg)

                    # ---- node MLP + residual ----
                    ps_z1 = PS.tile([24, N], F32, tag="pa")
                    nc.tensor.matmul(ps_z1,
                                     lhsT=nw1a_s[:, lay * 24:(lay + 1) * 24],
                                     rhs=normed, start=True, stop=False)
                    nc.tensor.matmul(ps_z1,
                                     lhsT=nw1b_s[:, lay * 24:(lay + 1) * 24],
                                     rhs=magg, start=False, stop=True)
                    s1 = LP.tile([24, N], F32, tag="s1")
                    nc.scalar.activation(s1, ps_z1, AF.Silu,
                                         bias=nb1_s[:, lay:lay + 1])
                    ps_z2 = PS.tile([D, N], F32, tag="pa")
                    nc.tensor.matmul(ps_z2,
                                     lhsT=nw2_s[:, lay * D:(lay + 1) * D],
                                     rhs=s1, start=True, stop=True)
                    feats_new = MP.tile([D, N], F32, tag="feats")
                    nc.vector.scalar_tensor_tensor(
                        out=feats_new, in0=ps_z2,
                        scalar=nb2_s[:, lay:lay + 1], in1=feats,
                        op0=ALU.add, op1=ALU.add)
                    feats = feats_new

                # ---- final head ----
                fmask = MP.tile([D, N], F32, tag="fmask")
                nc.vector.tensor_mul(fmask, feats, msk)
                ps_r = PS.tile([M, N], F32, tag="pa")
                nc.tensor.matmul(ps_r, lhsT=mw1_s, rhs=fmask,
                                 start=True, stop=True)
                r1 = MP.tile([M, N], F32, tag="r1")
                nc.scalar.activation(r1, ps_r, AF.Relu, bias=mb1_s[:, 0:1])
                ps_o = PS.tile([2, N], F32, tag="pa")
                nc.tensor.matmul(ps_o, lhsT=mw2_s, rhs=r1,
                                 start=True, stop=True)
                nc.vector.tensor_scalar_add(opad[:, :, 0:1], ps_o,
                                            mb2_s[:, 0:1])
                nc.sync.dma_start(
                    out=out[mol].rearrange("n c k -> c n k"), in_=opad)

    nc.finalize()
    return nc


_NC = None


def _get_nc():
    global _NC
    if _NC is None:
        _NC = build_nc()
    return _NC


def _bf(a):
    return np.ascontiguousarray(np.asarray(a, np.float32).astype(
        ml_dtypes.bfloat16))


def _prep_maps(x, mask, edge_w1, edge_b1, edge_w2, edge_b2, gate_w, gate_b,
               ln_g, ln_b, node_w1, node_b1, node_w2, node_b2,
               mlp_w1, mlp_b1, mlp_w2, mlp_b2):
    f = np.float32
    x = np.asarray(x, f)
    maskf = np.asarray(mask, f)
    ew1 = np.asarray(edge_w1, f)          # [L, 25, 50]
    eb1 = np.asarray(edge_b1, f)          # [L, 50]
    ew2 = np.asarray(edge_w2, f) * LIP    # [L, 50, 64]
    eb2 = np.asarray(edge_b2, f)          # [L, 64]
    gw = np.asarray(gate_w, f) * LIP      # [L, 64, 1]
    gb = np.asarray(gate_b, f)            # [L, 1]

    # chunks are i-pairs: the re-added per-j term uses W1fj, the
    # per-partition bias term uses W1fi (+b1)
    w1fi_h = np.transpose(ew1[:, D:2 * D, :], (1, 0, 2))   # [12, L, 50] (fj)
    w1fj_h = ew1[:, 0:D, :]                                # [L, 12, 50] (fi)
    w1d = ew1[:, 2 * D, :]                                 # [L, 50]

    w1fjE_h = np.zeros((D, L, 128), f)
    w1fjO_h = np.zeros((D, L, 128), f)
    b1pad_h = np.zeros((1, L, 128), f)
    w1d4q_h = np.zeros((4, L, 128), f)
    w2q_h = np.zeros((128, L, M), f)
    b2row_h = np.zeros((1, L, M), f)
    wgrep_h = np.zeros((128, L, 512), f)
    gbh_h = np.zeros((128, L), f)
    for l in range(L):
        w1fjE_h[:, l, 0:EH] = w1fj_h[l]
        w1fjO_h[:, l, Q:Q + EH] = w1fj_h[l]
        b1pad_h[0, l, 0:EH] = eb1[l]
        b1pad_h[0, l, Q:Q + EH] = eb1[l]
        w1d4q_h[0, l, 0:EH] = w1d[l]
        w1d4q_h[1, l, 0:EH] = w1d[l]
        w1d4q_h[2, l, Q:Q + EH] = w1d[l]
        w1d4q_h[3, l, Q:Q + EH] = w1d[l]
        w2q_h[0:EH, l, :] = ew2[l]
        w2q_h[Q:Q + EH, l, :] = ew2[l]
        b2row_h[0, l, :] = eb2[l]
        wgrep_h[:, l, :] = np.tile(gw[l, :, 0], (128, 8))
        gbh_h[:, l] = gb[l, 0] * 0.5

    ipad_h = np.zeros((EH, 128), f)
    for k in range(EH):
        ipad_h[k, k] = 1.0
        ipad_h[k, Q + k] = 1.0

    nw1 = np.asarray(node_w1, f)          # [L, 76, 24]
    nw1a_h = np.transpose(nw1[:, 0:D, :], (1, 0, 2))       # [12, L, 24]
    nw1b_h = np.transpose(nw1[:, D:, :] * (LIP * 0.5), (1, 0, 2))
    nw2_h = np.transpose(np.asarray(node_w2, f) * LIP, (1, 0, 2))

    parts = dict(
        w1fi=_bf(w1fi_h.reshape(D, L * EH)),
        w1fjE=_bf(w1fjE_h.reshape(D, L * 128)),
        w1fjO=_bf(w1fjO_h.reshape(D, L * 128)),
        b1pad=_bf(b1pad_h.reshape(1, L * 128)),
        w1d4q=_bf(w1d4q_h.reshape(4, L * 128)),
        w2q=_bf(w2q_h.reshape(128, L * M)),
        b2row=_bf(b2row_h.reshape(1, L * M)),
        wgrep=_bf(wgrep_h.reshape(128, L * 512)),
        ipad=_bf(ipad_h),
        i128b=_bf(np.eye(128, dtype=f)),
        c12=_bf(np.full((D, 1), 1.0 / D, f)),
    )
    partsf = dict(
        gbh=gbh_h,
        lng=np.asarray(ln_g, f).T,
        lnb=np.asarray(ln_b, f).T,
        nw1a=nw1a_h.reshape(D, L * 24),
        nw1b=nw1b_h.reshape(M, L * 24),
        nb1=np.asarray(node_b1, f).T,
        nw2=nw2_h.reshape(24, L * D),
        nb2=np.asarray(node_b2, f).T,
        mw1=np.asarray(mlp_w1, f),
        mb1=np.asarray(mlp_b1, f).reshape(M, 1),
        mw2=np.asarray(mlp_w2, f),
        mb2=np.asarray(mlp_b2, f).reshape(2, 1),
        i128f=np.eye(128, dtype=f),
    )
    wbf_h = np.zeros((128, WBF_X), ml_dtypes.bfloat16)
    for nm, p, w in WBF_SPEC:
        wbf_h[0:p, WBF_OFF[nm][2]:WBF_OFF[nm][2] + w] = parts[nm]
    wf32_h = np.zeros((128, WF32_X), f)
    for nm, p, w in WF32_SPEC:
        wf32_h[0:p, WF32_OFF[nm][2]:WF32_OFF[nm][2] + w] = partsf[nm]
    shared = dict(wbf=wbf_h, wf32=wf32_h)

    in_maps = []
    for core in range(NCORES):
        xs = x[core * BM:(core + 1) * BM]          # [2, 256, 6]
        feats0_h = np.zeros((BM, D, N), f)
        rd_h = np.zeros((BM, 4, 128, N), np.float32)
        m12 = np.zeros((BM, D, N), f)
        for m in range(BM):
            xm = xs[m]                              # [256, 6]
            feats0_h[m] = np.concatenate([xm, xm], axis=1).T
            nsq = np.sum(xm * xm, axis=1)           # [256]
            dmat = nsq[:, None] + nsq[None, :] - 2.0 * (xm @ xm.T)
            # rows (parity, hi/lo): rd[2p+q][c, i] = hi/lo of d(2c + p, i)
            dpc = dmat.reshape(128, 2, N).transpose(1, 0, 2)  # [p, c, i]
            dhi = dpc.astype(ml_dtypes.bfloat16).astype(np.float32)
            dlo = dpc - dhi
            rd_h[m, 0] = dhi[0]
            rd_h[m, 1] = dlo[0]
            rd_h[m, 2] = dhi[1]
            rd_h[m, 3] = dlo[1]
            m12[m] = np.broadcast_to(maskf[core * BM + m], (D, N))
        in_maps.append(dict(
            feats0=np.ascontiguousarray(feats0_h), rdin=_bf(rd_h),
            mask12=np.ascontiguousarray(m12),
            **{k: v.copy() for k, v in shared.items()},
        ))
    return in_maps


def kernel(**inputs):
    nc = _get_nc()
    in_maps = _prep_maps(**inputs)
    res = run_bass_kernel_spmd(nc, in_maps, core_ids=list(range(NCORES)))
    out = np.concatenate([r["out"] for r in res.results], axis=0)
    return out.astype(np.float32)

